# revision 11
# baseline (speedup 1.0000x reference)
"""FFT causal long-conv (H3/Hyena fftconv) as a blocked-Toeplitz matmul kernel
for 8 Trainium2 NeuronCores.

Math: y[b,d,l] = sum_{t<=l} filter[d,t] * x[b,d,l-t]  (causal conv, L taps).

Instead of an on-device FFT, the causal conv is computed directly as a
lower-block-triangular Toeplitz matmul: with 128-wide blocks (J=L/128 blocks),
y_i = sum_{k<=i} T_k @ x_{i-k} where T_k[a,c] = f[128k + a - c].  The T_k are
materialized host-side as PE-ready lhsT tiles, so the device does only
dense [128,128]x[128,N] matmuls accumulating in fp32 PSUM.

The dominant HBM traffic is the Toeplitz weight image (the filter replicated
128x at 1 MB/channel).  It is stored as float8e3 (E3M4, 4 mantissa bits) which
halves that traffic vs fp16; the tensor engine takes the fp8 lhsT directly
against an fp16 rhs at full bf16 rate.  The filter is pre-scaled by 64 into
e3m4's normal range and x is pre-scaled by 1/64, so no on-device dequant is
needed: (64 f) conv (x/64) = y exactly.

Sharding: channels D=1024 split 128 per core (each channel's conv is
independent); all B=16 batches stay on-core so each matmul gets the full
N=512 free dim.
"""

import numpy as np
import ml_dtypes


B, D, L = 16, 1024, 4096
NCORES = 8
DC = D // NCORES  # channels per core
C = 128           # time-block size == PE contraction dim
J = L // C        # 32 time blocks
N = J * B         # 512 = matmul free dim (j-block outer, batch inner)
GROUP = 4         # channels per DMA batch

F16 = np.float16
F8 = ml_dtypes.float8_e3m4
FSCALE = 64.0     # filter pre-scale into e3m4 normal range; x scaled by 1/64

_CACHE = {}


def _build_nc():
    if "nc" in _CACHE:
        return _CACHE["nc"]

    import concourse.bacc as bacc
    import concourse.tile as tile
    import concourse.mybir as mybir

    nc = bacc.Bacc("TRN2", target_bir_lowering=False, debug=False, num_devices=NCORES)

    # Layouts are chosen so every DMA has long contiguous per-partition runs:
    #   xt[c, d, n]    n = j*B + b         (input, time-within-block on partitions)
    #   ft[c, d, k, a] = f[d, 128k + a - c] (PE-ready lhsT Toeplitz tiles, e3m4)
    #   yt[a, d, n]    n = i*B + b         (output)
    xt = nc.dram_tensor("xt", [C, DC, N], mybir.dt.float16, kind="ExternalInput")
    ft = nc.dram_tensor("ft", [C, DC, J, C], mybir.dt.float8e3, kind="ExternalInput")
    yt = nc.dram_tensor("yt", [C, DC, N], mybir.dt.float16, kind="ExternalOutput")

    with tile.TileContext(nc) as tc:
        with (
            tc.tile_pool(name="wpool", bufs=3) as wpool,
            tc.tile_pool(name="xpool", bufs=3) as xpool,
            tc.tile_pool(name="ypool", bufs=3) as ypool,
            tc.tile_pool(name="pspool", bufs=7, space="PSUM") as pspool,
            tc.tile_pool(name="warmps", bufs=1, space="PSUM") as warmps,
        ):
            # The PE otherwise idles waiting for the first weight DMA and then
            # pays the HAM half-clock ramp (~3.4us of sustained activity to
            # trip K=8/8). A few dummy matmuls bridge until group 0's first
            # per-channel weight slice lands (~7us); the first real chains
            # finish the ramp as useful work.
            # K=1 stationary: the dummy matmuls only need a [1,128] zero tile,
            # so the memset they wait on is 128 elements and warmup starts as
            # early as the engine preamble allows.
            wz = wpool.tile([1, C], mybir.dt.float16, tag="warmz", bufs=1)
            nc.vector.memset(wz, 0.0)
            # 64 dummy matmuls x ~58ns cold = ~3.7us of sustained PE activity
            # ending right when the first weight/x semaphores fire (~10.5us,
            # the end-to-end DMA latency floor) -- trips the HAM clock gate to
            # K=8/8 so the real matmuls start at full 2.4 GHz.
            wps = warmps.tile([C, N], mybir.dt.float32)
            for _ in range(64):
                nc.tensor.matmul(wps[:, :64], wz[:, :C], wz[:, :64],
                                 start=True, stop=True)
            NG = DC // GROUP
            for g in range(NG):
                sl = slice(g * GROUP, (g + 1) * GROUP)
                # Keep both HWDGE rings (SP + ACT) continuously busy: each
                # group's weight load is split half/half across the rings.
                # Everything stays off the slow gpsimd SWDGE path.
                eng_a = nc.sync if g % 2 == 0 else nc.scalar
                eng_b = nc.scalar if g % 2 == 0 else nc.sync
                xg = xpool.tile([C, GROUP, N], mybir.dt.float16)
                wt = wpool.tile([C, GROUP, J, C], mybir.dt.float8e3)
                if g == 0:
                    # Finest-grained first loads, all gating transfers on the
                    # sync ring (its first semaphores fire ~0.5us before the
                    # scalar ring's): channel 0's x (128 KB) and the k<16 half
                    # of its weights (256 KB) gate the first real matmul.
                    eng_a.dma_start(out=xg[:, :1], in_=xt[:, :1, :])
                    eng_a.dma_start(out=wt[:, :1, :J // 2], in_=ft[:, :1, :J // 2, :])
                    eng_a.dma_start(out=wt[:, :1, J // 2:], in_=ft[:, :1, J // 2:, :])
                    eng_b.dma_start(out=xg[:, 1:], in_=xt[:, 1:GROUP, :])
                    eng_b.dma_start(out=wt[:, 1:2], in_=ft[:, 1:2, :, :])
                    eng_a.dma_start(out=wt[:, 2:3], in_=ft[:, 2:3, :, :])
                    eng_b.dma_start(out=wt[:, 3:4], in_=ft[:, 3:4, :, :])
                else:
                    eng_b.dma_start(out=xg, in_=xt[:, sl, :])
                    h = GROUP // 2
                    eng_a.dma_start(out=wt[:, :h], in_=ft[:, sl.start:sl.start + h, :, :])
                    eng_b.dma_start(out=wt[:, h:], in_=ft[:, sl.start + h:sl.stop, :, :])
                yg = ypool.tile([C, GROUP, N], mybir.dt.float16)
                for dd in range(GROUP):
                    ps = pspool.tile([C, N], mybir.dt.float32)
                    for k in range(J):
                        ncols = (J - k) * B
                        nc.tensor.matmul(
                            ps[:, k * B:],
                            wt[:, dd, k, :],
                            xg[:, dd, :ncols],
                            start=(k == 0),
                            stop=(k == J - 1),
                        )
                    last = g == NG - 1 and dd == GROUP - 1
                    if last:
                        # Drain the finished half of PSUM (cols [0,256) are
                        # complete after the k=15 matmul) while the tail
                        # matmuls still run, shortening the critical path
                        # from last matmul to last store.
                        nc.vector.tensor_copy(out=yg[:, dd, :C * 2],
                                              in_=ps[:, :C * 2])
                        eng_b.dma_start(out=yt[:, sl.start + dd, :C * 2],
                                        in_=yg[:, dd, :C * 2])
                        nc.vector.tensor_copy(out=yg[:, dd, C * 2:],
                                              in_=ps[:, C * 2:])
                        eng_b.dma_start(out=yt[:, sl.start + dd, C * 2:],
                                        in_=yg[:, dd, C * 2:])
                    else:
                        nc.vector.tensor_copy(out=yg[:, dd, :], in_=ps[:])
                        # Store each channel as soon as its PSUM drain
                        # finishes so the final store isn't serialized behind
                        # the whole group.
                        eng_b.dma_start(out=yt[:, sl.start + dd, :],
                                        in_=yg[:, dd, :])

    nc.compile()
    _CACHE["nc"] = nc
    return nc


def _prep_core_inputs(x, f, core):
    ds = slice(core * DC, (core + 1) * DC)
    xs = x[:, ds, :].reshape(B, DC, J, C).transpose(3, 1, 2, 0).reshape(C, DC, N)
    xt = np.ascontiguousarray(xs * (1.0 / FSCALE)).astype(F16)

    # Convert the (scaled) filter to e3m4 FIRST, then build the 128x
    # replicated Toeplitz image as a byte-level strided copy.
    # fpad[d, 127 + t] = f[d, t]; ft[c, d, m] = fpad[d, 127 + m - c]
    fpad = np.zeros((DC, 127 + L), dtype=F8)
    fpad[:, 127:] = (f[ds] * FSCALE).astype(F8)
    base = fpad[:, 127:]
    sv = np.lib.stride_tricks.as_strided(
        base,
        shape=(C, DC, L),
        strides=(-fpad.strides[1], fpad.strides[0], fpad.strides[1]),
    )
    ft = np.ascontiguousarray(sv).reshape(C, DC, J, C)
    return {"xt": xt, "ft": ft}


def _run(x, f, trace=False):
    from concourse.bass_utils import run_bass_kernel_spmd

    nc = _build_nc()
    in_maps = [_prep_core_inputs(x, f, i) for i in range(NCORES)]
    res = run_bass_kernel_spmd(
        nc, in_maps, core_ids=list(range(NCORES)), trace=trace
    )

    y = np.empty((B, D, L), dtype=np.float32)
    for i in range(NCORES):
        ytc = np.asarray(res.results[i]["yt"]).astype(np.float32)  # [C(a), DC, N]
        ys = ytc.reshape(C, DC, J, B).transpose(3, 1, 2, 0).reshape(B, DC, L)
        y[:, i * DC:(i + 1) * DC, :] = ys
    return y, res


def kernel(x, filter):
    x = np.asarray(x, dtype=np.float32)
    f = np.asarray(filter, dtype=np.float32)
    y, _ = _run(x, f, trace=False)
    return y


# revision 13
# speedup vs baseline: 1.0066x; 1.0066x over previous
"""FFT causal long-conv (H3/Hyena fftconv) as a blocked-Toeplitz matmul kernel
for 8 Trainium2 NeuronCores.

Math: y[b,d,l] = sum_{t<=l} filter[d,t] * x[b,d,l-t]  (causal conv, L taps).

Instead of an on-device FFT, the causal conv is computed directly as a
lower-block-triangular Toeplitz matmul: with 128-wide blocks (J=L/128 blocks),
y_i = sum_{k<=i} T_k @ x_{i-k} where T_k[a,c] = f[128k + a - c].  The T_k are
materialized host-side as PE-ready lhsT tiles, so the device does only
dense [128,128]x[128,N] matmuls accumulating in fp32 PSUM.

The dominant HBM traffic is the Toeplitz weight image (the filter replicated
128x at 1 MB/channel).  It is stored as float8e3 (E3M4, 4 mantissa bits) which
halves that traffic vs fp16; the tensor engine takes the fp8 lhsT directly
against an fp16 rhs at full bf16 rate.  The filter is pre-scaled by 64 into
e3m4's normal range and x is pre-scaled by 1/64, so no on-device dequant is
needed: (64 f) conv (x/64) = y exactly.

Sharding: channels D=1024 split 128 per core (each channel's conv is
independent); all B=16 batches stay on-core so each matmul gets the full
N=512 free dim.
"""

import numpy as np
import ml_dtypes


B, D, L = 16, 1024, 4096
NCORES = 8
DC = D // NCORES  # channels per core
C = 128           # time-block size == PE contraction dim
J = L // C        # 32 time blocks
N = J * B         # 512 = matmul free dim (j-block outer, batch inner)
GROUP = 4         # channels per DMA batch

F16 = np.float16
F8 = ml_dtypes.float8_e3m4
FSCALE = 64.0     # filter pre-scale into e3m4 normal range; x scaled by 1/64

_CACHE = {}


def _build_nc():
    if "nc" in _CACHE:
        return _CACHE["nc"]

    import concourse.bacc as bacc
    import concourse.tile as tile
    import concourse.mybir as mybir

    nc = bacc.Bacc("TRN2", target_bir_lowering=False, debug=False, num_devices=NCORES)

    # Layouts are chosen so every DMA has long contiguous per-partition runs:
    #   xt[c, d, n]    n = j*B + b         (input, time-within-block on partitions)
    #   ft[c, d, k, a] = f[d, 128k + a - c] (PE-ready lhsT Toeplitz tiles, e3m4)
    #   yt[a, d, n]    n = i*B + b         (output)
    xt = nc.dram_tensor("xt", [C, DC, N], mybir.dt.float16, kind="ExternalInput")
    ft = nc.dram_tensor("ft", [C, DC, J, C], mybir.dt.float8e3, kind="ExternalInput")
    yt = nc.dram_tensor("yt", [C, DC, N], mybir.dt.float16, kind="ExternalOutput")

    with tile.TileContext(nc) as tc:
        with (
            tc.tile_pool(name="wpool", bufs=3) as wpool,
            tc.tile_pool(name="xpool", bufs=3) as xpool,
            tc.tile_pool(name="ypool", bufs=3) as ypool,
            tc.tile_pool(name="pspool", bufs=7, space="PSUM") as pspool,
            tc.tile_pool(name="warmps", bufs=1, space="PSUM") as warmps,
        ):
            # The PE otherwise idles waiting for the first weight DMA and then
            # pays the HAM half-clock ramp (~3.4us of sustained activity to
            # trip K=8/8). A few dummy matmuls bridge until group 0's first
            # per-channel weight slice lands (~7us); the first real chains
            # finish the ramp as useful work.
            wz = wpool.tile([C, C], mybir.dt.float16, tag="warmz", bufs=1)
            nc.vector.memset(wz, 0.0)
            # 64 dummy matmuls x ~58ns cold = ~3.7us of sustained PE activity
            # ending right when the first weight/x semaphores fire (~10.7us,
            # the end-to-end DMA latency floor) -- trips the HAM clock gate to
            # K=8/8 so the real matmuls start at full 2.4 GHz.
            wps = warmps.tile([C, N], mybir.dt.float32)
            for _ in range(64):
                nc.tensor.matmul(wps[:, :64], wz[:, :C], wz[:, :64],
                                 start=True, stop=True)
            NG = DC // GROUP
            for g in range(NG):
                sl = slice(g * GROUP, (g + 1) * GROUP)
                # Keep both HWDGE rings (SP + ACT) continuously busy: each
                # group's weight load is split half/half across the rings.
                # Everything stays off the slow gpsimd SWDGE path.
                eng_a = nc.sync if g % 2 == 0 else nc.scalar
                eng_b = nc.scalar if g % 2 == 0 else nc.sync
                xg = xpool.tile([C, GROUP, N], mybir.dt.float16)
                wt = wpool.tile([C, GROUP, J, C], mybir.dt.float8e3)
                if g == 0:
                    # Finest-grained first loads, spread across both rings:
                    # channel 0's x (128 KB) and the k<16 half of its weights
                    # (256 KB) gate the first real matmul; everything else
                    # streams in behind them.
                    eng_b.dma_start(out=xg[:, :1], in_=xt[:, :1, :])
                    eng_a.dma_start(out=wt[:, :1, :J // 2], in_=ft[:, :1, :J // 2, :])
                    eng_a.dma_start(out=wt[:, :1, J // 2:], in_=ft[:, :1, J // 2:, :])
                    eng_b.dma_start(out=xg[:, 1:], in_=xt[:, 1:GROUP, :])
                    for dd in range(1, GROUP):
                        eng = eng_a if dd % 2 == 0 else eng_b
                        eng.dma_start(out=wt[:, dd:dd + 1],
                                      in_=ft[:, dd:dd + 1, :, :])
                else:
                    eng_b.dma_start(out=xg, in_=xt[:, sl, :])
                    h = GROUP // 2
                    eng_a.dma_start(out=wt[:, :h], in_=ft[:, sl.start:sl.start + h, :, :])
                    eng_b.dma_start(out=wt[:, h:], in_=ft[:, sl.start + h:sl.stop, :, :])
                yg = ypool.tile([C, GROUP, N], mybir.dt.float16)
                for dd in range(GROUP):
                    ps = pspool.tile([C, N], mybir.dt.float32)
                    for k in range(J):
                        ncols = (J - k) * B
                        nc.tensor.matmul(
                            ps[:, k * B:],
                            wt[:, dd, k, :],
                            xg[:, dd, :ncols],
                            start=(k == 0),
                            stop=(k == J - 1),
                        )
                    last = g == NG - 1 and dd == GROUP - 1
                    if last:
                        # Drain the finished half of PSUM (cols [0,256) are
                        # complete after the k=15 matmul) while the tail
                        # matmuls still run, shortening the critical path
                        # from last matmul to last store.
                        nc.vector.tensor_copy(out=yg[:, dd, :C * 2],
                                              in_=ps[:, :C * 2])
                        eng_b.dma_start(out=yt[:, sl.start + dd, :C * 2],
                                        in_=yg[:, dd, :C * 2])
                        nc.vector.tensor_copy(out=yg[:, dd, C * 2:],
                                              in_=ps[:, C * 2:])
                        eng_b.dma_start(out=yt[:, sl.start + dd, C * 2:],
                                        in_=yg[:, dd, C * 2:])
                    else:
                        nc.vector.tensor_copy(out=yg[:, dd, :], in_=ps[:])
                        # Store each channel as soon as its PSUM drain
                        # finishes so the final store isn't serialized behind
                        # the whole group.
                        eng_b.dma_start(out=yt[:, sl.start + dd, :],
                                        in_=yg[:, dd, :])

    nc.compile()
    _CACHE["nc"] = nc
    return nc


def _prep_core_inputs(x, f, core):
    ds = slice(core * DC, (core + 1) * DC)
    xs = x[:, ds, :].reshape(B, DC, J, C).transpose(3, 1, 2, 0).reshape(C, DC, N)
    xt = np.ascontiguousarray(xs * (1.0 / FSCALE)).astype(F16)

    # Convert the (scaled) filter to e3m4 FIRST, then build the 128x
    # replicated Toeplitz image as a byte-level strided copy.
    # fpad[d, 127 + t] = f[d, t]; ft[c, d, m] = fpad[d, 127 + m - c]
    fpad = np.zeros((DC, 127 + L), dtype=F8)
    fpad[:, 127:] = (f[ds] * FSCALE).astype(F8)
    base = fpad[:, 127:]
    sv = np.lib.stride_tricks.as_strided(
        base,
        shape=(C, DC, L),
        strides=(-fpad.strides[1], fpad.strides[0], fpad.strides[1]),
    )
    ft = np.ascontiguousarray(sv).reshape(C, DC, J, C)
    return {"xt": xt, "ft": ft}


def _run(x, f, trace=False):
    from concourse.bass_utils import run_bass_kernel_spmd

    nc = _build_nc()
    in_maps = [_prep_core_inputs(x, f, i) for i in range(NCORES)]
    res = run_bass_kernel_spmd(
        nc, in_maps, core_ids=list(range(NCORES)), trace=trace
    )

    y = np.empty((B, D, L), dtype=np.float32)
    for i in range(NCORES):
        ytc = np.asarray(res.results[i]["yt"]).astype(np.float32)  # [C(a), DC, N]
        ys = ytc.reshape(C, DC, J, B).transpose(3, 1, 2, 0).reshape(B, DC, L)
        y[:, i * DC:(i + 1) * DC, :] = ys
    return y, res


def kernel(x, filter):
    x = np.asarray(x, dtype=np.float32)
    f = np.asarray(filter, dtype=np.float32)
    y, _ = _run(x, f, trace=False)
    return y
